# revision 1
# baseline (speedup 1.0000x reference)
"""GCN edge-classifier kernel for Trainium2, 8 NeuronCores.

Math reduction: with NCLASS=2, softmax(logits)[e] = [sigmoid(d), 1-sigmoid(d)]
where d = du[col_e] + dw[row_e] + (bfc0-bfc1),
  du[v] = dinv[v]*(t_u[v] + a_u[v]) + b1@wu,   (wu = Wfc[:64,0]-Wfc[:64,1])
  a_u   = dinv * (x @ (W1 @ wu)),              (scalar per node)
  t_u[v]= sum_{edges e: col_e==v} a_u[row_e],
  dinv  = rsqrt(1 + indegree)
(and likewise *_w with ww = Wfc[64:,0]-Wfc[64:,1] on the row side).

Sharding: edges are sharded across the 8 cores by target (col) range of
12500 nodes, sorted by col, and packed into 128-node "windows" of fixed
slot count so all aggregation is window-static PE one-hot matmuls.
"""
import numpy as np

N = 100000
E = 1600000
NFEAT = 256
NSH = 12500           # nodes per core
NPAD = 12544          # 98 * 128
NW = 98               # 128-node windows per core
WS = 19               # columns (x128 slots) per window
NCOL = NW * WS        # 1862 columns per core
SLOTS = NCOL * 128    # 238336 slots per core
NFULL = NPAD * 8      # 100352
ZROW = NPAD * 7 + 12510   # a zeroed pad row (core 7 block) in translated ids

_compiled = None


def _build():
    import concourse.bass as bass
    import concourse.bacc as bacc
    import concourse.mybir as mybir
    from concourse.tile import TileContext, add_dep_helper
    from concourse.masks import make_identity

    AluOp = mybir.AluOpType
    Act = mybir.ActivationFunctionType
    f32 = mybir.dt.float32
    i32 = mybir.dt.int32

    nc = bacc.Bacc('TRN2', target_bir_lowering=False, debug=False, num_devices=8)

    # inputs
    xT = nc.dram_tensor('xT', [NFEAT, NPAD], f32, kind='ExternalInput')
    cwin = nc.dram_tensor('cwin', [128, NCOL], f32, kind='ExternalInput')   # window-local col, junk=999
    rt = nc.dram_tensor('rt', [128, NCOL], i32, kind='ExternalInput')       # translated row ids (pad=ZROW)
    W1 = nc.dram_tensor('W1', [NFEAT, 64], f32, kind='ExternalInput')
    Wfc = nc.dram_tensor('Wfc', [128, 2], f32, kind='ExternalInput')
    b1 = nc.dram_tensor('b1', [64, 1], f32, kind='ExternalInput')
    bfc = nc.dram_tensor('bfc', [1, 2], f32, kind='ExternalInput')
    out = nc.dram_tensor('out', [128, NCOL, 2], f32, kind='ExternalOutput')

    # internal DRAM
    A_loc = nc.dram_tensor('A_loc', [NPAD, 2], f32)
    A_full = nc.dram_tensor('A_full', [NFULL, 2], f32, addr_space='Shared')
    D_loc = nc.dram_tensor('D_loc', [NPAD, 2], f32)
    D_full = nc.dram_tensor('D_full', [NFULL, 2], f32, addr_space='Shared')

    with TileContext(nc) as tc:
        with tc.tile_pool(name='cst', bufs=1) as cst, \
             tc.tile_pool(name='ps', bufs=1, space='PSUM') as ps, \
             tc.tile_pool(name='psw', bufs=2, space='PSUM') as psw, \
             tc.tile_pool(name='big', bufs=1) as big, \
             tc.tile_pool(name='wrk', bufs=3) as wrk:

            ident = cst.tile([128, 128], f32)
            make_identity(nc, ident[:])

            # ---- constants: wuw [64,2], q chunks, cbc [128,2] ----
            wfct = cst.tile([128, 2], f32)
            nc.sync.dma_start(out=wfct[:], in_=Wfc[:, :])
            diff = cst.tile([128, 1], f32)
            nc.vector.tensor_tensor(out=diff[:], in0=wfct[:, 0:1], in1=wfct[:, 1:2], op=AluOp.subtract)
            wuw = cst.tile([64, 2], f32)
            nc.vector.tensor_copy(out=wuw[0:64, 0:1], in_=diff[0:64, 0:1])
            nc.sync.dma_start(out=wuw[0:64, 1:2], in_=diff[64:128, 0:1])

            # W1T [64, 256] via PE transpose
            w1a = cst.tile([128, 64], f32)
            w1b = cst.tile([128, 64], f32)
            nc.sync.dma_start(out=w1a[:], in_=W1[0:128, :])
            nc.sync.dma_start(out=w1b[:], in_=W1[128:256, :])
            w1t = cst.tile([64, 256], f32)
            pt = ps.tile([64, 128], f32, tag='cstp')
            nc.tensor.transpose(out=pt[:], in_=w1a[:], identity=ident[:])
            nc.vector.tensor_copy(out=w1t[:, 0:128], in_=pt[:])
            pt2 = ps.tile([64, 128], f32, tag='cstp')
            nc.tensor.transpose(out=pt2[:], in_=w1b[:], identity=ident[:])
            nc.vector.tensor_copy(out=w1t[:, 128:256], in_=pt2[:])

            # q = W1 @ wuw  -> q_lo/q_hi [128, 2]
            q_lo = cst.tile([128, 2], f32)
            q_hi = cst.tile([128, 2], f32)
            pq = ps.tile([128, 128], f32, tag='cstp')
            nc.tensor.matmul(out=pq[:, 0:2], lhsT=w1t[:, 0:128], rhs=wuw[:], start=True, stop=True)
            nc.vector.tensor_copy(out=q_lo[:], in_=pq[:, 0:2])
            pq2 = ps.tile([128, 128], f32, tag='cstp')
            nc.tensor.matmul(out=pq2[:, 0:2], lhsT=w1t[:, 128:256], rhs=wuw[:], start=True, stop=True)
            nc.vector.tensor_copy(out=q_hi[:], in_=pq2[:, 0:2])

            # beta = b1 @ wuw [1,2]; db = bfc0-bfc1; cuw = beta + [db, 0]
            b1t = cst.tile([64, 1], f32)
            nc.sync.dma_start(out=b1t[:], in_=b1[:, :])
            pb = ps.tile([128, 128], f32, tag='cstp')
            nc.tensor.matmul(out=pb[0:1, 0:2], lhsT=b1t[:], rhs=wuw[:], start=True, stop=True)
            bfct = cst.tile([1, 2], f32)
            nc.sync.dma_start(out=bfct[:], in_=bfc[:, :])
            cuw1 = cst.tile([1, 2], f32)
            nc.vector.tensor_copy(out=cuw1[:], in_=pb[0:1, 0:2])
            dbt = cst.tile([1, 1], f32)
            nc.vector.tensor_tensor(out=dbt[:], in0=bfct[0:1, 0:1], in1=bfct[0:1, 1:2], op=AluOp.subtract)
            nc.vector.tensor_tensor(out=cuw1[0:1, 0:1], in0=cuw1[0:1, 0:1], in1=dbt[:], op=AluOp.add)
            # broadcast to [128, 2] via ones matmul
            ones1 = cst.tile([1, 128], f32)
            nc.vector.memset(ones1[:], 1.0)
            pcb = ps.tile([128, 128], f32, tag='cstp')
            nc.tensor.matmul(out=pcb[:, 0:2], lhsT=ones1[:], rhs=cuw1[:], start=True, stop=True)
            cbc = cst.tile([128, 2], f32)
            nc.vector.tensor_copy(out=cbc[:], in_=pcb[:, 0:2])

            # iota row [128,128] f32: value = free index
            iotai = cst.tile([128, 128], i32)
            nc.gpsimd.iota(iotai[:], pattern=[[1, 128]], base=0, channel_multiplier=0)
            iotaf = cst.tile([128, 128], f32)
            nc.vector.tensor_copy(out=iotaf[:], in_=iotai[:])

            ones128 = cst.tile([128, 1], f32)
            nc.vector.memset(ones128[:], 1.0)

            # ---- xq matvec: v-minor tiles ----
            xlo = big.tile([128, NPAD], f32, tag='xlo')
            xhi = big.tile([128, NPAD], f32, tag='xhi')
            nc.sync.dma_start(out=xlo[:], in_=xT[0:128, :])
            nc.sync.dma_start(out=xhi[:], in_=xT[128:256, :])
            xq = big.tile([128, NW, 2], f32, tag='xq')
            for g in range(NW):
                pxq = psw.tile([128, 2], f32, tag='acc')
                nc.tensor.matmul(out=pxq[:], lhsT=xlo[:, 128 * g:128 * (g + 1)], rhs=q_lo[:], start=True, stop=False)
                nc.tensor.matmul(out=pxq[:], lhsT=xhi[:, 128 * g:128 * (g + 1)], rhs=q_hi[:], start=False, stop=True)
                nc.vector.tensor_copy(out=xq[:, g, :], in_=pxq[:])

            # ---- load edge streams ----
            cw_sb = big.tile([128, NCOL], f32, tag='cw')
            rt_sb = big.tile([128, NCOL], i32, tag='rt')
            nc.sync.dma_start(out=cw_sb[:], in_=cwin[:, :])
            nc.sync.dma_start(out=rt_sb[:], in_=rt[:, :])

            # ---- deg pass: windowed one-hot matmuls ----
            deg = big.tile([128, NW], f32, tag='deg')
            for g in range(NW):
                pdeg = psw.tile([128, 2], f32, tag='acc')
                for j in range(WS):
                    col = g * WS + j
                    oh = wrk.tile([128, 128], f32, tag='oh')
                    nc.vector.tensor_tensor(
                        out=oh[:], in0=cw_sb[:, col:col + 1].to_broadcast([128, 128]),
                        in1=iotaf[:], op=AluOp.is_equal)
                    nc.tensor.matmul(out=pdeg[:, 0:1], lhsT=oh[:], rhs=ones128[:],
                                     start=(j == 0), stop=(j == WS - 1))
                nc.vector.tensor_copy(out=deg[:, g:g + 1], in_=pdeg[:, 0:1])

            # ---- dinv, A table ----
            sq = wrk.tile([128, NW], f32, tag='sq')
            nc.scalar.activation(out=sq[:], in_=deg[:], func=Act.Sqrt, bias=1.0, scale=1.0)
            dinv = big.tile([128, NW], f32, tag='dinv')
            nc.vector.reciprocal(out=dinv[:], in_=sq[:])
            A_sb = big.tile([128, NW, 2], f32, tag='A')
            nc.vector.tensor_tensor(out=A_sb[:, :, 0], in0=xq[:, :, 0], in1=dinv[:], op=AluOp.mult)
            nc.vector.tensor_tensor(out=A_sb[:, :, 1], in0=xq[:, :, 1], in1=dinv[:], op=AluOp.mult)
            # pad nodes (>=12500) are already zero: xT pad cols are host-zeroed,
            # no edges touch them, so xq=0 and dinv=1 there.
            wA = nc.sync.dma_start(out=A_loc.rearrange('(f p) c -> p f c', p=128), in_=A_sb[:])
            cc1 = nc.gpsimd.collective_compute(
                'AllGather', AluOp.bypass, replica_groups=[list(range(8))],
                ins=[A_loc[:, :]], outs=[A_full[:, :]])
            add_dep_helper(cc1.ins, wA.ins, True, 'allgather after A write')

            # ---- t pass ----
            t_sb = big.tile([128, NW, 2], f32, tag='t')
            for g in range(NW):
                ptw = psw.tile([128, 2], f32, tag='acc')
                for j in range(WS):
                    col = g * WS + j
                    ap = wrk.tile([128, 2], f32, tag='ap')
                    gi = nc.gpsimd.indirect_dma_start(
                        out=ap[:], out_offset=None, in_=A_full[:, :],
                        in_offset=bass.IndirectOffsetOnAxis(ap=rt_sb[:, col:col + 1], axis=0))
                    add_dep_helper(gi.ins, cc1.ins, True, 'gather after allgather')
                    oh = wrk.tile([128, 128], f32, tag='oh')
                    nc.vector.tensor_tensor(
                        out=oh[:], in0=cw_sb[:, col:col + 1].to_broadcast([128, 128]),
                        in1=iotaf[:], op=AluOp.is_equal)
                    nc.tensor.matmul(out=ptw[:], lhsT=oh[:], rhs=ap[:],
                                     start=(j == 0), stop=(j == WS - 1))
                nc.vector.tensor_copy(out=t_sb[:, g, :], in_=ptw[:])

            # ---- D tables ----
            D_sb = big.tile([128, NW, 2], f32, tag='D')
            tmp = wrk.tile([128, NW], f32, tag='tmp')
            nc.vector.tensor_tensor(out=tmp[:], in0=t_sb[:, :, 0], in1=A_sb[:, :, 0], op=AluOp.add)
            nc.vector.tensor_tensor(out=tmp[:], in0=tmp[:], in1=dinv[:], op=AluOp.mult)
            nc.vector.tensor_scalar(out=D_sb[:, :, 0], in0=tmp[:], scalar1=cbc[:, 0:1], scalar2=None, op0=AluOp.add)
            tmp2 = wrk.tile([128, NW], f32, tag='tmp2')
            nc.vector.tensor_tensor(out=tmp2[:], in0=t_sb[:, :, 1], in1=A_sb[:, :, 1], op=AluOp.add)
            nc.vector.tensor_tensor(out=tmp2[:], in0=tmp2[:], in1=dinv[:], op=AluOp.mult)
            nc.vector.tensor_scalar(out=D_sb[:, :, 1], in0=tmp2[:], scalar1=cbc[:, 1:2], scalar2=None, op0=AluOp.add)
            wD = nc.sync.dma_start(out=D_loc.rearrange('(f p) c -> p f c', p=128), in_=D_sb[:])
            cc2 = nc.gpsimd.collective_compute(
                'AllGather', AluOp.bypass, replica_groups=[list(range(8))],
                ins=[D_loc[:, :]], outs=[D_full[:, :]])
            add_dep_helper(cc2.ins, wD.ins, True, 'allgather after D write')

            # ---- output pass ----
            for g in range(NW):
                ow = wrk.tile([128, WS, 2], f32, tag='ow')
                for j in range(WS):
                    col = g * WS + j
                    dp = wrk.tile([128, 2], f32, tag='dp')
                    gi = nc.gpsimd.indirect_dma_start(
                        out=dp[:], out_offset=None, in_=D_full[:, :],
                        in_offset=bass.IndirectOffsetOnAxis(ap=rt_sb[:, col:col + 1], axis=0))
                    add_dep_helper(gi.ins, cc2.ins, True, 'gather after allgather2')
                    # gu via transposed one-hot: psum_oht = transpose(onehot)
                    oh = wrk.tile([128, 128], f32, tag='oh')
                    nc.vector.tensor_tensor(
                        out=oh[:], in0=cw_sb[:, col:col + 1].to_broadcast([128, 128]),
                        in1=iotaf[:], op=AluOp.is_equal)
                    poht = psw.tile([128, 128], f32, tag='poht')
                    nc.tensor.transpose(out=poht[:], in_=oh[:], identity=ident[:])
                    oht = wrk.tile([128, 128], f32, tag='oht')
                    nc.vector.tensor_copy(out=oht[:], in_=poht[:])
                    pgu = psw.tile([128, 2], f32, tag='acc2')
                    nc.tensor.matmul(out=pgu[:, 0:1], lhsT=oht[:], rhs=D_sb[:, g, 0:1], start=True, stop=True)
                    # delta = gu + dw[row]
                    delta = wrk.tile([128, 1], f32, tag='delta')
                    nc.vector.tensor_tensor(out=delta[:], in0=pgu[:, 0:1], in1=dp[:, 1:2], op=AluOp.add)
                    nc.scalar.activation(out=ow[:, j, 0:1], in_=delta[:], func=Act.Sigmoid, scale=1.0)
                    nc.scalar.activation(out=ow[:, j, 1:2], in_=delta[:], func=Act.Sigmoid, scale=-1.0)
                nc.sync.dma_start(out=out[:, g * WS:(g + 1) * WS, :], in_=ow[:])

    nc.compile()
    return nc


def _pack(x, edge_index, W1, b1, Wfc, bfc):
    c = np.asarray(edge_index[1], dtype=np.int64)
    r = np.asarray(edge_index[0], dtype=np.int64)
    order = np.argsort(c, kind='stable')
    sc = c[order]
    sr = r[order]
    spos = order

    in_maps = []
    unpack = []   # (core, col, partition) -> original edge pos
    for k in range(8):
        lo, hi = np.searchsorted(sc, [k * NSH, (k + 1) * NSH])
        ck = sc[lo:hi] - k * NSH          # local col in [0, 12500)
        rk = sr[lo:hi]
        pk = spos[lo:hi]
        # window-local packing
        cw = np.full((128, NCOL), 999.0, dtype=np.float32)
        rtr = np.full((128, NCOL), ZROW, dtype=np.int32)
        posmap = np.full((128, NCOL), -1, dtype=np.int64)
        win = ck // 128
        # slot edges of window g into columns [g*WS, (g+1)*WS)
        wlo = np.searchsorted(win, np.arange(NW))
        whi = np.searchsorted(win, np.arange(NW), side='right')
        maxcnt = (whi - wlo).max()
        assert maxcnt <= WS * 128, f'window overflow: {maxcnt}'
        for g in range(NW):
            a, b = wlo[g], whi[g]
            n = b - a
            if n == 0:
                continue
            i = np.arange(n)
            pp = i % 128
            jj = g * WS + i // 128
            cw[pp, jj] = (ck[a:b] - g * 128).astype(np.float32)
            rtr[pp, jj] = (NPAD * (rk[a:b] // NSH) + rk[a:b] % NSH).astype(np.int32)
            posmap[pp, jj] = pk[a:b]
        xk = np.zeros((NFEAT, NPAD), dtype=np.float32)
        xk[:, :NSH] = np.asarray(x[k * NSH:(k + 1) * NSH], dtype=np.float32).T
        in_maps.append({
            'xT': xk, 'cwin': cw, 'rt': rtr,
            'W1': np.asarray(W1, np.float32),
            'Wfc': np.asarray(Wfc, np.float32),
            'b1': np.asarray(b1, np.float32).reshape(64, 1),
            'bfc': np.asarray(bfc, np.float32).reshape(1, 2),
        })
        unpack.append(posmap)
    return in_maps, unpack


def kernel(x, edge_index, W1, b1, Wfc, bfc):
    global _compiled
    from concourse import bass_utils
    in_maps, unpack = _pack(x, edge_index, W1, b1, Wfc, bfc)
    if _compiled is None:
        _compiled = _build()
    res = bass_utils.run_bass_kernel_spmd(_compiled, in_maps, core_ids=list(range(8)))
    out = np.zeros((E, 2), dtype=np.float32)
    for k in range(8):
        o = res.results[k]['out']          # [128, NCOL, 2]
        pm = unpack[k]                     # [128, NCOL]
        mask = pm >= 0
        out[pm[mask]] = o[mask]
    return out



# revision 7
# speedup vs baseline: 3.4740x; 3.4740x over previous
"""GCN edge-classifier kernel for Trainium2, 8 NeuronCores.

Math reduction: with NCLASS=2, softmax(logits)[e] = [sigmoid(d), 1-sigmoid(d)]
where d = D0[col_e] + D1[row_e],
  D0[v] = dinv[v]*(t_u[v] + A_u[v]) + (b1@wu + bfc0-bfc1),
  D1[v] = dinv[v]*(t_w[v] + A_w[v]) + (b1@ww),
  A[v]  = dinv[v] * (x[v] @ (W1 @ [wu|ww])),     (2 scalars per node)
  t[v]  = sum_{edges e: col_e==v} A[row_e],
  dinv  = rsqrt(1 + indegree),
  wu = Wfc[:64,0]-Wfc[:64,1], ww = Wfc[64:,0]-Wfc[64:,1].

Sharding: edges sharded across 8 cores by target (col) range of 12500 nodes.
Per core, nodes are ranked by descending in-degree; window g = ranks
[128g, 128(g+1)) across the 128 partitions. Each node's incoming edges sit
contiguously in its partition's row at columns [colstart[g], colstart[g]+deg),
padded to the window-common width K[g] with slots pointing at a zeroed table
row. Aggregation is a plain per-window tensor_reduce along the free axis.
Cross-node fetches use per-column indirect DMA (128 rows / instruction, the
hardware's limit: one offset per partition), from bf16 node tables that are
allgathered between the passes.
"""
import numpy as np

N = 100000
E = 1600000
NFEAT = 256
NSH = 12500           # nodes per core
NW = 98               # 128-rank windows per core
NPAD = NW * 128       # 12544
NFULL = NPAD * 8      # 100352
ZROW_LOCAL = 12543    # pad rank on every core; A row (p=127)*98+(g=97)
ZROW = NPAD * 7 + ZROW_LOCAL

_compiled = None
_compiled_key = None
_meta = None          # (Ks, colstart, NCOLT) from the last _pack


def _build(Ks):
    import concourse.bass as bass
    import concourse.bacc as bacc
    import concourse.mybir as mybir
    from concourse.tile import TileContext, add_dep_helper
    from concourse.masks import make_identity

    AluOp = mybir.AluOpType
    Act = mybir.ActivationFunctionType
    f32 = mybir.dt.float32
    bf16 = mybir.dt.bfloat16
    i32 = mybir.dt.int32

    colstart = np.concatenate([[0], np.cumsum(Ks)]).astype(int)
    NCOLT = int(colstart[-1])

    nc = bacc.Bacc('TRN2', target_bir_lowering=False, debug=False, num_devices=8,
                   num_swdge_queues=4)

    # inputs
    xT = nc.dram_tensor('xT', [NFEAT, NPAD], bf16, kind='ExternalInput')
    rt = nc.dram_tensor('rt', [128, NCOLT], i32, kind='ExternalInput')   # translated row ids (pad=ZROW)
    W1 = nc.dram_tensor('W1', [NFEAT, 64], f32, kind='ExternalInput')
    Wfc = nc.dram_tensor('Wfc', [128, 2], f32, kind='ExternalInput')
    b1 = nc.dram_tensor('b1', [64, 1], f32, kind='ExternalInput')
    bfc = nc.dram_tensor('bfc', [1, 2], f32, kind='ExternalInput')
    out = nc.dram_tensor('out', [2, 128, NCOLT], f32, kind='ExternalOutput')

    # internal DRAM node tables (bf16); row of node with rank q on core k is
    # k*NPAD + (q%128)*98 + q//98-free layout (partition-major: p*NW + g) so
    # the table DMA is one contiguous run per partition.
    A_loc = nc.dram_tensor('A_loc', [NPAD, 2], bf16)
    A_full = nc.dram_tensor('A_full', [NFULL, 2], bf16, addr_space='Shared')
    D_loc = nc.dram_tensor('D_loc', [NPAD, 2], bf16)
    D_full = nc.dram_tensor('D_full', [NFULL, 2], bf16, addr_space='Shared')

    with TileContext(nc) as tc:
        with tc.tile_pool(name='cst', bufs=1) as cst, \
             tc.tile_pool(name='ps', bufs=1, space='PSUM') as ps, \
             tc.tile_pool(name='psw', bufs=4, space='PSUM') as psw, \
             tc.tile_pool(name='big', bufs=1) as big, \
             tc.tile_pool(name='wrk', bufs=2) as wrk:

            # ---- big loads first: rt gates the deg phase ----
            rt_sb = big.tile([128, NCOLT], i32, tag='rt')
            nc.sync.dma_start(out=rt_sb[:], in_=rt[:, :])
            xlo = big.tile([128, NPAD], bf16, tag='xlo')
            xhi = big.tile([128, NPAD], bf16, tag='xhi')
            nc.sync.dma_start(out=xlo[:], in_=xT[0:128, :])
            nc.scalar.dma_start(out=xhi[:], in_=xT[128:256, :])

            ident = cst.tile([128, 128], f32)
            make_identity(nc, ident[:])

            # ---- constants: wuw [64,2] = [wu|ww] ----
            wfct = cst.tile([128, 2], f32)
            nc.sync.dma_start(out=wfct[:], in_=Wfc[:, :])
            diff = cst.tile([128, 1], f32)
            nc.vector.tensor_tensor(out=diff[:], in0=wfct[:, 0:1], in1=wfct[:, 1:2], op=AluOp.subtract)
            wuw = cst.tile([64, 2], f32)
            nc.vector.tensor_copy(out=wuw[0:64, 0:1], in_=diff[0:64, 0:1])
            nc.sync.dma_start(out=wuw[0:64, 1:2], in_=diff[64:128, 0:1])

            # W1T [64, 256] via PE transpose
            w1a = cst.tile([128, 64], f32)
            w1b = cst.tile([128, 64], f32)
            nc.sync.dma_start(out=w1a[:], in_=W1[0:128, :])
            nc.sync.dma_start(out=w1b[:], in_=W1[128:256, :])
            w1t = cst.tile([64, 256], f32)
            pt = ps.tile([64, 128], f32, tag='cstp')
            nc.tensor.transpose(out=pt[:], in_=w1a[:], identity=ident[:])
            nc.vector.tensor_copy(out=w1t[:, 0:128], in_=pt[:])
            pt2 = ps.tile([64, 128], f32, tag='cstp')
            nc.tensor.transpose(out=pt2[:], in_=w1b[:], identity=ident[:])
            nc.vector.tensor_copy(out=w1t[:, 128:256], in_=pt2[:])

            # q = W1 @ wuw  -> bf16 q_lo/q_hi [128, 2] for the bf16 matvec
            q_lo = cst.tile([128, 2], bf16)
            q_hi = cst.tile([128, 2], bf16)
            pq = ps.tile([128, 128], f32, tag='cstp')
            nc.tensor.matmul(out=pq[:, 0:2], lhsT=w1t[:, 0:128], rhs=wuw[:], start=True, stop=True)
            nc.vector.tensor_copy(out=q_lo[:], in_=pq[:, 0:2])
            pq2 = ps.tile([128, 128], f32, tag='cstp')
            nc.tensor.matmul(out=pq2[:, 0:2], lhsT=w1t[:, 128:256], rhs=wuw[:], start=True, stop=True)
            nc.vector.tensor_copy(out=q_hi[:], in_=pq2[:, 0:2])

            # cbc [128,2]: col 0 = b1@wu + (bfc0-bfc1), col 1 = b1@ww
            b1t = cst.tile([64, 1], f32)
            nc.sync.dma_start(out=b1t[:], in_=b1[:, :])
            pb = ps.tile([128, 128], f32, tag='cstp')
            nc.tensor.matmul(out=pb[0:1, 0:2], lhsT=b1t[:], rhs=wuw[:], start=True, stop=True)
            bfct = cst.tile([1, 2], f32)
            nc.sync.dma_start(out=bfct[:], in_=bfc[:, :])
            cuw1 = cst.tile([1, 2], f32)
            nc.vector.tensor_copy(out=cuw1[:], in_=pb[0:1, 0:2])
            dbt = cst.tile([1, 1], f32)
            nc.vector.tensor_tensor(out=dbt[:], in0=bfct[0:1, 0:1], in1=bfct[0:1, 1:2], op=AluOp.subtract)
            nc.vector.tensor_tensor(out=cuw1[0:1, 0:1], in0=cuw1[0:1, 0:1], in1=dbt[:], op=AluOp.add)
            ones1 = cst.tile([1, 128], f32)
            nc.vector.memset(ones1[:], 1.0)
            pcb = ps.tile([128, 128], f32, tag='cstp')
            nc.tensor.matmul(out=pcb[:, 0:2], lhsT=ones1[:], rhs=cuw1[:], start=True, stop=True)
            cbc = cst.tile([128, 2], f32)
            nc.vector.tensor_copy(out=cbc[:], in_=pcb[:, 0:2])

            # ---- deg from pad mask ----
            rtf = wrk.tile([128, NCOLT], f32, tag='rtf')
            nc.vector.tensor_copy(out=rtf[:], in_=rt_sb[:])
            mask = wrk.tile([128, NCOLT], f32, tag='mask')
            nc.vector.tensor_scalar(out=mask[:], in0=rtf[:], scalar1=float(ZROW),
                                    scalar2=None, op0=AluOp.not_equal)
            deg = big.tile([128, NW], f32, tag='deg')
            nc.vector.memset(deg[:], 0.0)
            for g in range(NW):
                c0, c1 = int(colstart[g]), int(colstart[g + 1])
                if c1 > c0:
                    nc.vector.tensor_reduce(out=deg[:, g:g + 1], in_=mask[:, c0:c1],
                                            axis=mybir.AxisListType.X, op=AluOp.add)
            sq = wrk.tile([128, NW], f32, tag='sq')
            nc.scalar.activation(out=sq[:], in_=deg[:], func=Act.Sqrt, bias=1.0, scale=1.0)
            dinv = big.tile([128, NW], f32, tag='dinv')
            nc.vector.reciprocal(out=dinv[:], in_=sq[:])

            # ---- A = dinv * (x @ q), per 128-rank window; bf16 table copy ----
            A_sb = big.tile([128, NW, 2], f32, tag='A')
            A_bf = big.tile([128, NW, 2], bf16, tag='Abf')
            for g in range(NW):
                pxq = psw.tile([128, 2], f32, tag='acc')
                nc.tensor.matmul(out=pxq[:], lhsT=xlo[:, 128 * g:128 * (g + 1)], rhs=q_lo[:], start=True, stop=False)
                nc.tensor.matmul(out=pxq[:], lhsT=xhi[:, 128 * g:128 * (g + 1)], rhs=q_hi[:], start=False, stop=True)
                nc.vector.tensor_tensor(out=A_sb[:, g, :], in0=pxq[:],
                                        in1=dinv[:, g:g + 1].to_broadcast([128, 2]), op=AluOp.mult)
            nc.vector.tensor_copy(out=A_bf[:], in_=A_sb[:])
            wA = nc.sync.dma_start(out=A_loc.rearrange('(p f) c -> p f c', p=128), in_=A_bf[:])
            cc1 = nc.gpsimd.collective_compute(
                'AllGather', AluOp.bypass, replica_groups=[list(range(8))],
                ins=[A_loc[:, :]], outs=[A_full[:, :]])
            add_dep_helper(cc1.ins, wA.ins, True, 'allgather after A write')

            # ---- pass 2: per-column gather of A[row], reduce per window ----
            ap_big = big.tile([128, NCOLT, 2], bf16, tag='ap')
            for c in range(NCOLT):
                gi = nc.gpsimd.indirect_dma_start(
                    out=ap_big[:, c, :], out_offset=None, in_=A_full[:, :],
                    in_offset=bass.IndirectOffsetOnAxis(ap=rt_sb[:, c:c + 1], axis=0))
                add_dep_helper(gi.ins, cc1.ins, True, 'gather after allgather')
            t_sb = big.tile([128, NW, 2], f32, tag='t')
            nc.vector.memset(t_sb[:], 0.0)
            for g in range(NW):
                c0, c1 = int(colstart[g]), int(colstart[g + 1])
                if c1 > c0:
                    nc.vector.tensor_reduce(
                        out=t_sb[:, g, :], in_=ap_big[:, c0:c1, :].rearrange('p k c -> p c k'),
                        axis=mybir.AxisListType.X, op=AluOp.add)

            # ---- D tables ----
            D_sb = big.tile([128, NW, 2], f32, tag='D')
            D_bf = big.tile([128, NW, 2], bf16, tag='Dbf')
            nc.vector.tensor_tensor(out=D_sb[:], in0=t_sb[:], in1=A_sb[:], op=AluOp.add)
            for ch in range(2):
                nc.vector.tensor_tensor(out=D_sb[:, :, ch], in0=D_sb[:, :, ch], in1=dinv[:], op=AluOp.mult)
                nc.vector.tensor_scalar(out=D_sb[:, :, ch], in0=D_sb[:, :, ch],
                                        scalar1=cbc[:, ch:ch + 1], scalar2=None, op0=AluOp.add)
            nc.vector.tensor_copy(out=D_bf[:], in_=D_sb[:])
            wD = nc.sync.dma_start(out=D_loc.rearrange('(p f) c -> p f c', p=128), in_=D_bf[:])
            cc2 = nc.gpsimd.collective_compute(
                'AllGather', AluOp.bypass, replica_groups=[list(range(8))],
                ins=[D_loc[:, :]], outs=[D_full[:, :]])
            add_dep_helper(cc2.ins, wD.ins, True, 'allgather after D write')

            # ---- pass 3: per-column gather of D[row], add local D0, sigmoid ----
            dp_big = big.tile([128, NCOLT, 2], bf16, tag='dp')
            for c in range(NCOLT):
                gi = nc.gpsimd.indirect_dma_start(
                    out=dp_big[:, c, :], out_offset=None, in_=D_full[:, :],
                    in_offset=bass.IndirectOffsetOnAxis(ap=rt_sb[:, c:c + 1], axis=0))
                add_dep_helper(gi.ins, cc2.ins, True, 'gather after allgather2')
            z = big.tile([128, NCOLT], f32, tag='z')
            for g in range(NW):
                c0, c1 = int(colstart[g]), int(colstart[g + 1])
                if c1 > c0:
                    nc.vector.tensor_scalar(out=z[:, c0:c1], in0=dp_big[:, c0:c1, 1],
                                            scalar1=D_sb[:, g, 0:1], scalar2=None, op0=AluOp.add)
            ow0 = big.tile([128, NCOLT], f32, tag='ow0')
            ow1 = big.tile([128, NCOLT], f32, tag='ow1')
            nc.scalar.activation(out=ow0[:], in_=z[:], func=Act.Sigmoid, scale=1.0)
            nc.scalar.activation(out=ow1[:], in_=z[:], func=Act.Sigmoid, scale=-1.0)
            nc.sync.dma_start(out=out[0, :, :], in_=ow0[:])
            nc.scalar.dma_start(out=out[1, :, :], in_=ow1[:])

    nc.compile()
    return nc


def _pack(x, edge_index, W1, b1, Wfc, bfc):
    global _meta
    r = np.asarray(edge_index[0], dtype=np.int64)
    c = np.asarray(edge_index[1], dtype=np.int64)
    deg_all = np.bincount(c, minlength=N)

    # per-core degree-descending rank; translated table row per node
    pos = np.empty(N, dtype=np.int64)
    rank_of = np.empty(N, dtype=np.int64)
    Ks_cores = np.zeros((8, NW), dtype=np.int64)
    orders = []
    for k in range(8):
        d = deg_all[k * NSH:(k + 1) * NSH]
        order = np.argsort(-d, kind='stable')
        orders.append(order)
        rank = np.empty(NSH, dtype=np.int64)
        rank[order] = np.arange(NSH)
        rank_of[k * NSH:(k + 1) * NSH] = rank
        pos[k * NSH:(k + 1) * NSH] = k * NPAD + (rank % 128) * NW + rank // 128
        sd = d[order]
        for g in range(NW):
            lo = g * 128
            if lo < NSH:
                Ks_cores[k, g] = sd[lo]
    Ks = [int(v) for v in Ks_cores.max(axis=0)]
    colstart = np.concatenate([[0], np.cumsum(Ks)]).astype(int)
    NCOLT = int(colstart[-1])
    _meta = (tuple(Ks), colstart, NCOLT)

    order_e = np.argsort(c, kind='stable')
    sc = c[order_e]
    sr = r[order_e]
    spos = order_e

    in_maps = []
    unpack = []
    for k in range(8):
        lo, hi = np.searchsorted(sc, [k * NSH, (k + 1) * NSH])
        ck = sc[lo:hi]                     # global col ids, sorted
        rk = sr[lo:hi]
        pk = spos[lo:hi]
        # j = index of the edge within its node's contiguous run
        run_start = np.searchsorted(ck, ck, side='left')
        j = np.arange(len(ck)) - run_start
        rank = rank_of[ck]
        g = rank // 128
        p = rank % 128
        col = colstart[g] + j
        rtr = np.full((128, NCOLT), ZROW, dtype=np.int32)
        posmap = np.full((128, NCOLT), -1, dtype=np.int64)
        rtr[p, col] = pos[rk].astype(np.int32)
        posmap[p, col] = pk
        # x in rank order (column index == rank), bf16, pad tail zero
        xk = np.zeros((NFEAT, NPAD), dtype=np.float32)
        xk[:, :NSH] = np.asarray(x[k * NSH:(k + 1) * NSH], dtype=np.float32)[orders[k]].T
        import ml_dtypes
        xk = xk.astype(ml_dtypes.bfloat16)
        in_maps.append({
            'xT': xk, 'rt': rtr,
            'W1': np.asarray(W1, np.float32),
            'Wfc': np.asarray(Wfc, np.float32),
            'b1': np.asarray(b1, np.float32).reshape(64, 1),
            'bfc': np.asarray(bfc, np.float32).reshape(1, 2),
        })
        unpack.append(posmap)
    return in_maps, unpack


def kernel(x, edge_index, W1, b1, Wfc, bfc):
    global _compiled, _compiled_key
    from concourse import bass_utils
    in_maps, unpack = _pack(x, edge_index, W1, b1, Wfc, bfc)
    Ks, colstart, NCOLT = _meta
    if _compiled is None or _compiled_key != Ks:
        _compiled = _build(list(Ks))
        _compiled_key = Ks
    res = bass_utils.run_bass_kernel_spmd(_compiled, in_maps, core_ids=list(range(8)))
    out = np.zeros((E, 2), dtype=np.float32)
    for k in range(8):
        o = res.results[k]['out']          # [2, 128, NCOLT]
        pm = unpack[k]
        m = pm >= 0
        out[pm[m], 0] = o[0][m]
        out[pm[m], 1] = o[1][m]
    return out


# revision 8
# speedup vs baseline: 3.8941x; 1.1209x over previous
"""GCN edge-classifier kernel for Trainium2, 8 NeuronCores.

Math reduction: with NCLASS=2, softmax(logits)[e] = [sigmoid(d), 1-sigmoid(d)]
where d = D0[col_e] + D1[row_e],
  D0[v] = dinv[v]*(t_u[v] + A_u[v]) + (b1@wu + bfc0-bfc1),
  D1[v] = dinv[v]*(t_w[v] + A_w[v]) + (b1@ww),
  A[v]  = dinv[v] * (x[v] @ (W1 @ [wu|ww])),     (2 scalars per node)
  t[v]  = sum_{edges e: col_e==v} A[row_e],
  dinv  = rsqrt(1 + indegree),
  wu = Wfc[:64,0]-Wfc[:64,1], ww = Wfc[64:,0]-Wfc[64:,1].

Sharding: edges sharded across 8 cores by target (col) range of 12500 nodes.
Per core, nodes are ranked by descending in-degree; window g = ranks
[128g, 128(g+1)) across the 128 partitions. Each node's incoming edges sit
contiguously in its partition's row at columns [colstart[g], colstart[g]+deg),
padded to the window-common width K[g] with slots pointing at a zeroed table
row. Aggregation is a plain per-window tensor_reduce along the free axis.
Cross-node fetches use per-column indirect DMA (128 rows / instruction, the
hardware's limit: one offset per partition), from bf16 node tables that are
allgathered between the passes.
"""
import numpy as np

N = 100000
E = 1600000
NFEAT = 256
NSH = 12500           # nodes per core
NW = 98               # 128-rank windows per core
NPAD = NW * 128       # 12544
NFULL = NPAD * 8      # 100352
ZROW_LOCAL = 12543    # pad rank on every core; A row (p=127)*98+(g=97)
ZROW = NPAD * 7 + ZROW_LOCAL

_compiled = None
_compiled_key = None
_meta = None          # (Ks, colstart, NCOLT) from the last _pack


def _build(Ks):
    import concourse.bass as bass
    import concourse.bacc as bacc
    import concourse.mybir as mybir
    from concourse.tile import TileContext, add_dep_helper
    from concourse.masks import make_identity

    AluOp = mybir.AluOpType
    Act = mybir.ActivationFunctionType
    f32 = mybir.dt.float32
    bf16 = mybir.dt.bfloat16
    i32 = mybir.dt.int32

    colstart = np.concatenate([[0], np.cumsum(Ks)]).astype(int)
    NCOLT = int(colstart[-1])

    nc = bacc.Bacc('TRN2', target_bir_lowering=False, debug=False, num_devices=8,
                   num_swdge_queues=4)

    # inputs
    xT = nc.dram_tensor('xT', [NFEAT, NPAD], bf16, kind='ExternalInput')
    rt = nc.dram_tensor('rt', [128, NCOLT], i32, kind='ExternalInput')   # translated row ids (pad=ZROW)
    W1 = nc.dram_tensor('W1', [NFEAT, 64], f32, kind='ExternalInput')
    Wfc = nc.dram_tensor('Wfc', [128, 2], f32, kind='ExternalInput')
    b1 = nc.dram_tensor('b1', [64, 1], f32, kind='ExternalInput')
    bfc = nc.dram_tensor('bfc', [1, 2], f32, kind='ExternalInput')
    out = nc.dram_tensor('out', [2, 128, NCOLT], f32, kind='ExternalOutput')

    # internal DRAM node tables (bf16); row of node with rank q on core k is
    # k*NPAD + (q%128)*NW + q//128 (partition-major) so the table write DMA
    # is one contiguous run per partition.
    A_loc = nc.dram_tensor('A_loc', [NPAD, 2], bf16)
    A_full = nc.dram_tensor('A_full', [NFULL, 2], bf16, addr_space='Shared')
    D_loc = nc.dram_tensor('D_loc', [NPAD, 2], bf16)
    D_full = nc.dram_tensor('D_full', [NFULL, 2], bf16, addr_space='Shared')

    with TileContext(nc) as tc:
        with tc.tile_pool(name='cst', bufs=1) as cst, \
             tc.tile_pool(name='ps', bufs=1, space='PSUM') as ps, \
             tc.tile_pool(name='psw', bufs=4, space='PSUM') as psw, \
             tc.tile_pool(name='big', bufs=1) as big, \
             tc.tile_pool(name='wrk', bufs=2) as wrk:

            # ---- big loads first: rt gates the deg phase ----
            rt_sb = big.tile([128, NCOLT], i32, tag='rt')
            nc.sync.dma_start(out=rt_sb[:], in_=rt[:, :])
            xlo = big.tile([128, NPAD], bf16, tag='xlo')
            xhi = big.tile([128, NPAD], bf16, tag='xhi')
            nc.sync.dma_start(out=xlo[:], in_=xT[0:128, :])
            nc.scalar.dma_start(out=xhi[:], in_=xT[128:256, :])

            ident = cst.tile([128, 128], f32)
            make_identity(nc, ident[:])

            # ---- constants: wuw [64,2] = [wu|ww] ----
            wfct = cst.tile([128, 2], f32)
            nc.sync.dma_start(out=wfct[:], in_=Wfc[:, :])
            diff = cst.tile([128, 1], f32)
            nc.vector.tensor_tensor(out=diff[:], in0=wfct[:, 0:1], in1=wfct[:, 1:2], op=AluOp.subtract)
            wuw = cst.tile([64, 2], f32)
            nc.vector.tensor_copy(out=wuw[0:64, 0:1], in_=diff[0:64, 0:1])
            nc.sync.dma_start(out=wuw[0:64, 1:2], in_=diff[64:128, 0:1])

            # W1T [64, 256] via PE transpose
            w1a = cst.tile([128, 64], f32)
            w1b = cst.tile([128, 64], f32)
            nc.sync.dma_start(out=w1a[:], in_=W1[0:128, :])
            nc.sync.dma_start(out=w1b[:], in_=W1[128:256, :])
            w1t = cst.tile([64, 256], f32)
            pt = ps.tile([64, 128], f32, tag='cstp')
            nc.tensor.transpose(out=pt[:], in_=w1a[:], identity=ident[:])
            nc.vector.tensor_copy(out=w1t[:, 0:128], in_=pt[:])
            pt2 = ps.tile([64, 128], f32, tag='cstp')
            nc.tensor.transpose(out=pt2[:], in_=w1b[:], identity=ident[:])
            nc.vector.tensor_copy(out=w1t[:, 128:256], in_=pt2[:])

            # q = W1 @ wuw  -> bf16 q_lo/q_hi [128, 2] for the bf16 matvec
            q_lo = cst.tile([128, 2], bf16)
            q_hi = cst.tile([128, 2], bf16)
            pq = ps.tile([128, 128], f32, tag='cstp')
            nc.tensor.matmul(out=pq[:, 0:2], lhsT=w1t[:, 0:128], rhs=wuw[:], start=True, stop=True)
            nc.vector.tensor_copy(out=q_lo[:], in_=pq[:, 0:2])
            pq2 = ps.tile([128, 128], f32, tag='cstp')
            nc.tensor.matmul(out=pq2[:, 0:2], lhsT=w1t[:, 128:256], rhs=wuw[:], start=True, stop=True)
            nc.vector.tensor_copy(out=q_hi[:], in_=pq2[:, 0:2])

            # cbc [128,2]: col 0 = b1@wu + (bfc0-bfc1), col 1 = b1@ww
            b1t = cst.tile([64, 1], f32)
            nc.sync.dma_start(out=b1t[:], in_=b1[:, :])
            pb = ps.tile([128, 128], f32, tag='cstp')
            nc.tensor.matmul(out=pb[0:1, 0:2], lhsT=b1t[:], rhs=wuw[:], start=True, stop=True)
            bfct = cst.tile([1, 2], f32)
            nc.sync.dma_start(out=bfct[:], in_=bfc[:, :])
            cuw1 = cst.tile([1, 2], f32)
            nc.vector.tensor_copy(out=cuw1[:], in_=pb[0:1, 0:2])
            dbt = cst.tile([1, 1], f32)
            nc.vector.tensor_tensor(out=dbt[:], in0=bfct[0:1, 0:1], in1=bfct[0:1, 1:2], op=AluOp.subtract)
            nc.vector.tensor_tensor(out=cuw1[0:1, 0:1], in0=cuw1[0:1, 0:1], in1=dbt[:], op=AluOp.add)
            ones1 = cst.tile([1, 128], f32)
            nc.vector.memset(ones1[:], 1.0)
            pcb = ps.tile([128, 128], f32, tag='cstp')
            nc.tensor.matmul(out=pcb[:, 0:2], lhsT=ones1[:], rhs=cuw1[:], start=True, stop=True)
            cbc = cst.tile([128, 2], f32)
            nc.vector.tensor_copy(out=cbc[:], in_=pcb[:, 0:2])

            # ---- deg from pad mask ----
            rtf = wrk.tile([128, NCOLT], f32, tag='rtf')
            nc.vector.tensor_copy(out=rtf[:], in_=rt_sb[:])
            mask = wrk.tile([128, NCOLT], f32, tag='mask')
            nc.vector.tensor_scalar(out=mask[:], in0=rtf[:], scalar1=float(ZROW),
                                    scalar2=None, op0=AluOp.not_equal)
            deg = big.tile([128, NW], f32, tag='deg')
            nc.vector.memset(deg[:], 0.0)
            for g in range(NW):
                c0, c1 = int(colstart[g]), int(colstart[g + 1])
                if c1 > c0:
                    nc.vector.tensor_reduce(out=deg[:, g:g + 1], in_=mask[:, c0:c1],
                                            axis=mybir.AxisListType.X, op=AluOp.add)
            sq = wrk.tile([128, NW], f32, tag='sq')
            nc.scalar.activation(out=sq[:], in_=deg[:], func=Act.Sqrt, bias=1.0, scale=1.0)
            dinv = big.tile([128, NW], f32, tag='dinv')
            nc.vector.reciprocal(out=dinv[:], in_=sq[:])

            # ---- A = dinv * (x @ q), per 128-rank window; bf16 table copy ----
            A_sb = big.tile([128, NW, 2], f32, tag='A')
            A_bf = big.tile([128, NW, 2], bf16, tag='Abf')
            for g in range(NW):
                pxq = psw.tile([128, 2], f32, tag='acc')
                nc.tensor.matmul(out=pxq[:], lhsT=xlo[:, 128 * g:128 * (g + 1)], rhs=q_lo[:], start=True, stop=False)
                nc.tensor.matmul(out=pxq[:], lhsT=xhi[:, 128 * g:128 * (g + 1)], rhs=q_hi[:], start=False, stop=True)
                nc.vector.tensor_tensor(out=A_sb[:, g, :], in0=pxq[:],
                                        in1=dinv[:, g:g + 1].to_broadcast([128, 2]), op=AluOp.mult)
            nc.vector.tensor_copy(out=A_bf[:], in_=A_sb[:])
            wA = nc.sync.dma_start(out=A_loc.rearrange('(p f) c -> p f c', p=128), in_=A_bf[:])
            cc1 = nc.gpsimd.collective_compute(
                'AllGather', AluOp.bypass, replica_groups=[list(range(8))],
                ins=[A_loc[:, :]], outs=[A_full[:, :]])
            add_dep_helper(cc1.ins, wA.ins, True, 'allgather after A write')

            # ---- pass 2: per-column gather of A[row], reduce per window ----
            ap_big = big.tile([128, NCOLT, 2], bf16, tag='ap')
            for c in range(NCOLT):
                gi = nc.gpsimd.indirect_dma_start(
                    out=ap_big[:, c, :], out_offset=None, in_=A_full[:, :],
                    in_offset=bass.IndirectOffsetOnAxis(ap=rt_sb[:, c:c + 1], axis=0))
                add_dep_helper(gi.ins, cc1.ins, True, 'gather after allgather')
            t_sb = big.tile([128, NW, 2], f32, tag='t')
            nc.vector.memset(t_sb[:], 0.0)
            for g in range(NW):
                c0, c1 = int(colstart[g]), int(colstart[g + 1])
                if c1 > c0:
                    nc.vector.tensor_reduce(
                        out=t_sb[:, g, :], in_=ap_big[:, c0:c1, :].rearrange('p k c -> p c k'),
                        axis=mybir.AxisListType.X, op=AluOp.add)

            # ---- D tables ----
            D_sb = big.tile([128, NW, 2], f32, tag='D')
            D_bf = big.tile([128, NW, 2], bf16, tag='Dbf')
            nc.vector.tensor_tensor(out=D_sb[:], in0=t_sb[:], in1=A_sb[:], op=AluOp.add)
            for ch in range(2):
                nc.vector.tensor_tensor(out=D_sb[:, :, ch], in0=D_sb[:, :, ch], in1=dinv[:], op=AluOp.mult)
                nc.vector.tensor_scalar(out=D_sb[:, :, ch], in0=D_sb[:, :, ch],
                                        scalar1=cbc[:, ch:ch + 1], scalar2=None, op0=AluOp.add)
            nc.vector.tensor_copy(out=D_bf[:], in_=D_sb[:])
            wD = nc.sync.dma_start(out=D_loc.rearrange('(p f) c -> p f c', p=128), in_=D_bf[:])
            cc2 = nc.gpsimd.collective_compute(
                'AllGather', AluOp.bypass, replica_groups=[list(range(8))],
                ins=[D_loc[:, :]], outs=[D_full[:, :]])
            add_dep_helper(cc2.ins, wD.ins, True, 'allgather after D write')

            # ---- pass 3: per-column gather of D[row], add local D0, sigmoid ----
            dp_big = big.tile([128, NCOLT, 2], bf16, tag='dp')
            for c in range(NCOLT):
                gi = nc.gpsimd.indirect_dma_start(
                    out=dp_big[:, c, :], out_offset=None, in_=D_full[:, :],
                    in_offset=bass.IndirectOffsetOnAxis(ap=rt_sb[:, c:c + 1], axis=0))
                add_dep_helper(gi.ins, cc2.ins, True, 'gather after allgather2')
            z = big.tile([128, NCOLT], f32, tag='z')
            for g in range(NW):
                c0, c1 = int(colstart[g]), int(colstart[g + 1])
                if c1 > c0:
                    nc.vector.tensor_scalar(out=z[:, c0:c1], in0=dp_big[:, c0:c1, 1],
                                            scalar1=D_sb[:, g, 0:1], scalar2=None, op0=AluOp.add)
            ow0 = big.tile([128, NCOLT], f32, tag='ow0')
            ow1 = big.tile([128, NCOLT], f32, tag='ow1')
            nc.scalar.activation(out=ow0[:], in_=z[:], func=Act.Sigmoid, scale=1.0)
            nc.scalar.activation(out=ow1[:], in_=z[:], func=Act.Sigmoid, scale=-1.0)
            nc.sync.dma_start(out=out[0, :, :], in_=ow0[:])
            nc.scalar.dma_start(out=out[1, :, :], in_=ow1[:])

    nc.compile()
    return nc


def _pack(x, edge_index, W1, b1, Wfc, bfc):
    global _meta
    r = np.asarray(edge_index[0], dtype=np.int64)
    c = np.asarray(edge_index[1], dtype=np.int64)
    deg_all = np.bincount(c, minlength=N)

    # per-core degree-descending rank; translated table row per node
    pos = np.empty(N, dtype=np.int64)
    rank_of = np.empty(N, dtype=np.int64)
    Ks_cores = np.zeros((8, NW), dtype=np.int64)
    orders = []
    for k in range(8):
        d = deg_all[k * NSH:(k + 1) * NSH]
        order = np.argsort(-d, kind='stable')
        orders.append(order)
        rank = np.empty(NSH, dtype=np.int64)
        rank[order] = np.arange(NSH)
        rank_of[k * NSH:(k + 1) * NSH] = rank
        pos[k * NSH:(k + 1) * NSH] = k * NPAD + (rank % 128) * NW + rank // 128
        sd = d[order]
        for g in range(NW):
            lo = g * 128
            if lo < NSH:
                Ks_cores[k, g] = sd[lo]
    Ks = [int(v) for v in Ks_cores.max(axis=0)]
    colstart = np.concatenate([[0], np.cumsum(Ks)]).astype(int)
    NCOLT = int(colstart[-1])
    _meta = (tuple(Ks), colstart, NCOLT)

    order_e = np.argsort(c, kind='stable')
    sc = c[order_e]
    sr = r[order_e]
    spos = order_e

    in_maps = []
    unpack = []
    for k in range(8):
        lo, hi = np.searchsorted(sc, [k * NSH, (k + 1) * NSH])
        ck = sc[lo:hi]                     # global col ids, sorted
        rk = sr[lo:hi]
        pk = spos[lo:hi]
        # j = index of the edge within its node's contiguous run
        run_start = np.searchsorted(ck, ck, side='left')
        j = np.arange(len(ck)) - run_start
        rank = rank_of[ck]
        g = rank // 128
        p = rank % 128
        col = colstart[g] + j
        rtr = np.full((128, NCOLT), ZROW, dtype=np.int32)
        posmap = np.full((128, NCOLT), -1, dtype=np.int64)
        rtr[p, col] = pos[rk].astype(np.int32)
        posmap[p, col] = pk
        # x in rank order (column index == rank), bf16, pad tail zero
        xk = np.zeros((NFEAT, NPAD), dtype=np.float32)
        xk[:, :NSH] = np.asarray(x[k * NSH:(k + 1) * NSH], dtype=np.float32)[orders[k]].T
        import ml_dtypes
        xk = xk.astype(ml_dtypes.bfloat16)
        in_maps.append({
            'xT': xk, 'rt': rtr,
            'W1': np.asarray(W1, np.float32),
            'Wfc': np.asarray(Wfc, np.float32),
            'b1': np.asarray(b1, np.float32).reshape(64, 1),
            'bfc': np.asarray(bfc, np.float32).reshape(1, 2),
        })
        unpack.append(posmap)
    return in_maps, unpack


def kernel(x, edge_index, W1, b1, Wfc, bfc):
    global _compiled, _compiled_key
    from concourse import bass_utils
    in_maps, unpack = _pack(x, edge_index, W1, b1, Wfc, bfc)
    Ks, colstart, NCOLT = _meta
    if _compiled is None or _compiled_key != Ks:
        _compiled = _build(list(Ks))
        _compiled_key = Ks
    res = bass_utils.run_bass_kernel_spmd(_compiled, in_maps, core_ids=list(range(8)))
    out = np.zeros((E, 2), dtype=np.float32)
    for k in range(8):
        o = res.results[k]['out']          # [2, 128, NCOLT]
        pm = unpack[k]
        m = pm >= 0
        out[pm[m], 0] = o[0][m]
        out[pm[m], 1] = o[1][m]
    return out
